# revision 12
# baseline (speedup 1.0000x reference)
"""Fused GEMM + bias + logsumexp + 2x leaky_relu + 2x exact-gelu kernel for TRN2.

Problem: x:(32768,2048)f16, W:(2048,2048)f16, bias:(2048,)f16
  y = x @ W + bias            (M, N)
  z = logsumexp(y, axis=1)    (M, 1)
  z = leaky_relu(leaky_relu(z, 0.01), 0.01)
  z = gelu(gelu(z, exact))    -> (M, 1) f16

Sharding: data-parallel over M across 8 cores (4096 rows each); W and bias
replicated. No cross-core communication; logsumexp reduces over N locally.

Per-core structure:
- W lives whole in SBUF (16 per-k chunk DMAs on the scalar HWDGE ring so the
  first matmuls only wait for chunk 0; the x DMA-transposes ride the sync ring
  in parallel — transposes are issued for super-block 0 *before* the W copies
  because Tile serializes transpose-after-copy, not copy-after-transpose).
- x arrives via DMA-transpose in 512-row super-blocks as 16 per-k xT tiles
  [128k x 512m], double-buffered.
- Per 128-row m-tile: 64 matmuls ([128,128]x[128,512] fp16, 16 k-steps x 4
  psum banks), then 4 fused DVE tensor_tensor_reduce ops that compute
  yneg = -(psum+bias) (f16) while min-accumulating negmax across banks, then
  one ACT Exp pass (scale=-1, bias=negmax) with accumulated row-sum.
- negmax/sumexp land in per-m-tile columns of [128, 32] stats tiles; the
  whole logsumexp tail (ln, +max, lrelu^2, erf-gelu^2) runs once, batched,
  at the end (keeps the ACT table on Exp for the entire main loop).
- The [128,32] result is block-transposed on DVE to [32,128] so the final
  store writes 256B-contiguous DRAM runs instead of 4096 scattered elements.
"""

import numpy as np

import concourse.bass as bass
import concourse.tile as tile
from concourse import bacc, mybir
from concourse.bass_utils import run_bass_kernel_spmd
from concourse.masks import make_identity

M, K, N = 32768, 2048, 2048
N_CORES = 8
M_SHARD = M // N_CORES  # 4096
P = 128
FREE = 512              # matmul moving free dim = one PSUM bank of f32
KT = K // P             # 16 k-subtiles
NB = N // FREE          # 4 psum banks per m-tile

f16 = mybir.dt.float16
f32 = mybir.dt.float32
AF = mybir.ActivationFunctionType
ALU = mybir.AluOpType

SQRT1_2 = 0.7071067811865476
ERF_CLIP = 5.9  # erf(5.9) == 1.0 to fp32 precision; clamp keeps ACT table in range


def build_program(m_shard=M_SHARD, num_devices=N_CORES, transposes_first=True):
    nc = bacc.Bacc(
        "TRN2",
        target_bir_lowering=False,
        debug=False,
        enable_asserts=False,
        num_devices=num_devices,
    )
    x = nc.dram_tensor("x", [m_shard, K], f16, kind="ExternalInput").ap()
    W = nc.dram_tensor("W", [K, N], f16, kind="ExternalInput").ap()
    bias = nc.dram_tensor("bias", [N], f16, kind="ExternalInput").ap()
    out = nc.dram_tensor("out", [m_shard, 1], f16, kind="ExternalOutput").ap()

    SBL = 512 if m_shard % 512 == 0 else P  # super-block rows per xT load
    MI = SBL // P                           # m-tiles per super-block
    NSB = m_shard // SBL                    # super-blocks
    MT = m_shard // P                       # total m-tiles

    with tile.TileContext(nc) as tc:
        with (
            tc.tile_pool(name="wpool", bufs=1) as wpool,
            tc.tile_pool(name="xpool", bufs=2) as xpool,
            tc.tile_pool(name="epool", bufs=2) as epool,
            tc.tile_pool(name="spool", bufs=4) as spool,
            tc.tile_pool(name="opool", bufs=1) as opool,
            tc.tile_pool(name="pspool", bufs=8, space="PSUM") as pspool,
        ):
            def issue_transposes(sb):
                xts = []
                for k in range(KT):
                    xk = xpool.tile([P, SBL], f16, tag=f"xk{k}", name=f"xT{sb}_{k}")
                    nc.sync.dma_start_transpose(
                        xk[:], x[bass.ds(sb * SBL, SBL), bass.ts(k, P)]
                    )
                    xts.append(xk)
                return xts

            # super-block 0 transposes FIRST (Tile serializes the first
            # transpose after a plain copy, but not the reverse), then the
            # W/bias copies on the other (scalar) HWDGE ring.
            def issue_weight_loads():
                bias_sb = wpool.tile([P, N], f16, name="bias_sb")
                nc.scalar.dma_start(bias_sb[:], bias[None, :].to_broadcast((P, N)))
                Wks = []
                for k in range(KT):
                    wk = wpool.tile([P, N], f16, tag=f"W{k}", name=f"W{k}")
                    nc.scalar.dma_start(wk[:], W[bass.ts(k, P), :])
                    Wks.append(wk)
                return bias_sb, Wks

            if transposes_first:
                xts = issue_transposes(0)
                bias_sb, Wks = issue_weight_loads()
            else:
                bias_sb, Wks = issue_weight_loads()
                xts = issue_transposes(0)

            nm_all = opool.tile([P, MT], f32)  # -rowmax per m-tile column
            se_all = opool.tile([P, MT], f32)  # sum(exp(y-max)) per m-tile column

            for sb in range(NSB):
                if sb > 0:
                    xts = issue_transposes(sb)
                for mi in range(MI):
                    t = sb * MI + mi
                    pss = [
                        pspool.tile([P, FREE], f32, tag="ps", name=f"ps{t}_{nb}")
                        for nb in range(NB)
                    ]
                    for k in range(KT):
                        lhsT = xts[k][:, bass.ts(mi, P)]
                        for nb in range(NB):
                            nc.tensor.matmul(
                                pss[nb][:],
                                lhsT,
                                Wks[k][:, bass.ts(nb, FREE)],
                                start=(k == 0),
                                stop=(k == KT - 1),
                            )
                    # y = psum + bias in f16 (matches the reference's fp16 GEMM
                    # output); negmax via negated row-max reduce.
                    # (tensor_tensor_reduce would fuse these but faults on HW.)
                    y = epool.tile([P, N], f16, tag="yneg", name=f"y{t}")
                    for nb in range(NB):
                        nc.vector.tensor_tensor(
                            y[:, bass.ts(nb, FREE)],
                            pss[nb][:],
                            bias_sb[:, bass.ts(nb, FREE)],
                            ALU.add,
                        )
                    nc.vector.reduce_max(
                        nm_all[:, t : t + 1],
                        y[:, :],
                        axis=mybir.AxisListType.X,
                        negate=True,
                    )
                    # exp(y - max); row-sum via the ACT accumulator
                    ejunk = epool.tile([P, N], f16, tag="ejunk", name=f"ejunk{t}")
                    nc.scalar.activation(
                        ejunk[:],
                        y[:],
                        AF.Exp,
                        bias=nm_all[:, t : t + 1],
                        accum_out=se_all[:, t : t + 1],
                    )

            # ---- batched tail over all MT m-tiles: [128, MT] ----
            z = opool.tile([P, MT], f32)
            nc.scalar.activation(z[:], se_all[:], AF.Ln)
            nc.vector.tensor_tensor(z[:], z[:], nm_all[:], ALU.subtract)  # +max
            w1 = opool.tile([P, MT], f32)
            for _ in range(2):  # leaky_relu = max(z, 0.01 z)
                nc.vector.tensor_scalar_mul(w1[:], z[:], 0.01)
                nc.vector.tensor_tensor(z[:], z[:], w1[:], ALU.max)
            for _ in range(2):  # gelu(z) = 0.5 z (1 + erf(z/sqrt(2)))
                u = opool.tile([P, MT], f32, tag="u")
                nc.vector.tensor_scalar(
                    u[:], z[:], SQRT1_2, ERF_CLIP, ALU.mult, ALU.min
                )
                nc.vector.tensor_scalar_max(u[:], u[:], -ERF_CLIP)
                e = opool.tile([P, MT], f32, tag="e")
                nc.scalar.activation(e[:], u[:], AF.Erf)
                nc.vector.tensor_tensor(e[:], z[:], e[:], ALU.mult)
                nc.vector.tensor_tensor(z[:], z[:], e[:], ALU.add)
                nc.vector.tensor_scalar_mul(z[:], z[:], 0.5)
            z16 = opool.tile([P, MT], f16)
            nc.vector.tensor_copy(z16[:], z[:])

            # PE-transpose [128, MT] -> [MT, 128] (PE is idle by now) so the
            # final store writes 256B-contiguous DRAM runs per partition.
            ident = opool.tile([P, P], f16, name="ident")
            make_identity(nc, ident[:])
            # reuse a "ps" slot (same 2KB/partition footprint; all matmul use
            # of the tag is over by now)
            psT = pspool.tile([MT, 2 * FREE], f16, tag="ps", name="pst")
            nc.tensor.transpose(psT[:, :P], z16[:], ident[:])
            outT = opool.tile([MT, P], f16, name="outT")
            nc.vector.tensor_copy(outT[:], psT[:, :P])
            nc.sync.dma_start(out.rearrange("(t p) o -> t (p o)", p=P), outT[:])

    nc.compile()
    return nc


_prog_cache = {}
LAST_RESULTS = None


def kernel(x, W, bias):
    global LAST_RESULTS
    x = np.ascontiguousarray(x)
    W = np.ascontiguousarray(W)
    bias = np.ascontiguousarray(bias)
    assert x.shape == (M, K) and W.shape == (K, N) and bias.shape == (N,)

    key = (M_SHARD, N_CORES)
    if key not in _prog_cache:
        _prog_cache[key] = build_program(*key)
    nc = _prog_cache[key]

    shards = np.split(x, N_CORES, axis=0)
    in_maps = [{"x": s, "W": W, "bias": bias} for s in shards]
    res = run_bass_kernel_spmd(nc, in_maps, list(range(N_CORES)))
    LAST_RESULTS = res
    return np.concatenate([res.results[i]["out"] for i in range(N_CORES)], axis=0)


# revision 14
# speedup vs baseline: 1.0920x; 1.0920x over previous
"""Fused GEMM + bias + logsumexp + 2x leaky_relu + 2x exact-gelu kernel for TRN2.

Problem: x:(32768,2048)f16, W:(2048,2048)f16, bias:(2048,)f16
  y = x @ W + bias            (M, N)
  z = logsumexp(y, axis=1)    (M, 1)
  z = leaky_relu(leaky_relu(z, 0.01), 0.01)
  z = gelu(gelu(z, exact))    -> (M, 1) f16

Sharding: data-parallel over M across 8 cores (4096 rows each); W and bias
replicated. No cross-core communication; logsumexp reduces over N locally.

Per-core structure:
- W lives whole in SBUF (16 per-k chunk DMAs on the scalar HWDGE ring so the
  first matmuls only wait for chunk 0; the x DMA-transposes ride the sync ring
  in parallel — transposes are issued for super-block 0 *before* the W copies
  because Tile serializes transpose-after-copy, not copy-after-transpose).
- x arrives via DMA-transpose in 512-row super-blocks as 16 per-k xT tiles
  [128k x 512m], double-buffered.
- Per 128-row m-tile: 64 matmuls ([128,128]x[128,512] fp16, 16 k-steps x 4
  psum banks), then 4 fused DVE tensor_tensor_reduce ops that compute
  yneg = -(psum+bias) (f16) while min-accumulating negmax across banks, then
  one ACT Exp pass (scale=-1, bias=negmax) with accumulated row-sum.
- negmax/sumexp land in per-m-tile columns of [128, 32] stats tiles; the
  whole logsumexp tail (ln, +max, lrelu^2, erf-gelu^2) runs once, batched,
  at the end (keeps the ACT table on Exp for the entire main loop).
- The [128,32] result is block-transposed on DVE to [32,128] so the final
  store writes 256B-contiguous DRAM runs instead of 4096 scattered elements.
"""

import numpy as np

import concourse.bass as bass
import concourse.tile as tile
from concourse import bacc, mybir
from concourse.bass_utils import run_bass_kernel_spmd
from concourse.masks import make_identity

M, K, N = 32768, 2048, 2048
N_CORES = 8
M_SHARD = M // N_CORES  # 4096
P = 128
FREE = 512              # matmul moving free dim = one PSUM bank of f32
KT = K // P             # 16 k-subtiles
NB = N // FREE          # 4 psum banks per m-tile

f16 = mybir.dt.float16
f32 = mybir.dt.float32
AF = mybir.ActivationFunctionType
ALU = mybir.AluOpType

SQRT1_2 = 0.7071067811865476
ERF_CLIP = 5.9  # erf(5.9) == 1.0 to fp32 precision; clamp keeps ACT table in range


def build_program(m_shard=M_SHARD, num_devices=N_CORES, transposes_first=True):
    nc = bacc.Bacc(
        "TRN2",
        target_bir_lowering=False,
        debug=False,
        enable_asserts=False,
        num_devices=num_devices,
    )
    x = nc.dram_tensor("x", [m_shard, K], f16, kind="ExternalInput").ap()
    W = nc.dram_tensor("W", [K, N], f16, kind="ExternalInput").ap()
    bias = nc.dram_tensor("bias", [N], f16, kind="ExternalInput").ap()
    out = nc.dram_tensor("out", [m_shard, 1], f16, kind="ExternalOutput").ap()

    SBL = 512 if m_shard % 512 == 0 else P  # super-block rows per xT load
    MI = SBL // P                           # m-tiles per super-block
    NSB = m_shard // SBL                    # super-blocks
    MT = m_shard // P                       # total m-tiles

    with tile.TileContext(nc) as tc:
        with (
            tc.tile_pool(name="wpool", bufs=1) as wpool,
            tc.tile_pool(name="xpool", bufs=2) as xpool,
            tc.tile_pool(name="epool", bufs=2) as epool,
            tc.tile_pool(name="spool", bufs=4) as spool,
            tc.tile_pool(name="opool", bufs=1) as opool,
            tc.tile_pool(name="pspool", bufs=8, space="PSUM") as pspool,
        ):
            def issue_transposes(sb):
                xts = []
                for k in range(KT):
                    xk = xpool.tile([P, SBL], f16, tag=f"xk{k}", name=f"xT{sb}_{k}")
                    nc.sync.dma_start_transpose(
                        xk[:], x[bass.ds(sb * SBL, SBL), bass.ts(k, P)]
                    )
                    xts.append(xk)
                return xts

            # super-block 0 transposes FIRST (Tile serializes the first
            # transpose after a plain copy, but not the reverse). W and bias
            # go on the SWDGE (gpsimd) ring: it has its own completion
            # semaphore lanes, so the big W load doesn't interleave into the
            # 8 HWDGE lanes the transposes round-robin on.
            def issue_weight_loads():
                W_sb = wpool.tile([P, KT, N], f16, name="W_sb")
                nc.gpsimd.dma_start(W_sb[:], W.rearrange("(ko p) n -> p ko n", p=P))
                bias_sb = wpool.tile([P, N], f16, name="bias_sb")
                nc.gpsimd.dma_start(bias_sb[:], bias[None, :].to_broadcast((P, N)))
                return bias_sb, W_sb

            if transposes_first:
                xts = issue_transposes(0)
                bias_sb, W_sb = issue_weight_loads()
            else:
                bias_sb, W_sb = issue_weight_loads()
                xts = issue_transposes(0)

            nm_all = opool.tile([P, MT], f32)  # -rowmax per m-tile column
            se_all = opool.tile([P, MT], f32)  # sum(exp(y-max)) per m-tile column

            for sb in range(NSB):
                if sb > 0:
                    xts = issue_transposes(sb)
                for mi in range(MI):
                    t = sb * MI + mi
                    pss = [
                        pspool.tile([P, FREE], f32, tag="ps", name=f"ps{t}_{nb}")
                        for nb in range(NB)
                    ]
                    for k in range(KT):
                        lhsT = xts[k][:, bass.ts(mi, P)]
                        for nb in range(NB):
                            nc.tensor.matmul(
                                pss[nb][:],
                                lhsT,
                                W_sb[:, k, bass.ts(nb, FREE)],
                                start=(k == 0),
                                stop=(k == KT - 1),
                            )
                    # y = psum + bias in f16 (matches the reference's fp16 GEMM
                    # output); negmax via negated row-max reduce.
                    # (tensor_tensor_reduce would fuse these but faults on HW.)
                    y = epool.tile([P, N], f16, tag="yneg", name=f"y{t}")
                    for nb in range(NB):
                        nc.vector.tensor_tensor(
                            y[:, bass.ts(nb, FREE)],
                            pss[nb][:],
                            bias_sb[:, bass.ts(nb, FREE)],
                            ALU.add,
                        )
                    nc.vector.reduce_max(
                        nm_all[:, t : t + 1],
                        y[:, :],
                        axis=mybir.AxisListType.X,
                        negate=True,
                    )
                    # exp(y - max); row-sum via the ACT accumulator
                    ejunk = epool.tile([P, N], f16, tag="ejunk", name=f"ejunk{t}")
                    nc.scalar.activation(
                        ejunk[:],
                        y[:],
                        AF.Exp,
                        bias=nm_all[:, t : t + 1],
                        accum_out=se_all[:, t : t + 1],
                    )

            # ---- batched tail over all MT m-tiles: [128, MT] ----
            z = opool.tile([P, MT], f32)
            nc.scalar.activation(z[:], se_all[:], AF.Ln)
            nc.vector.tensor_tensor(z[:], z[:], nm_all[:], ALU.subtract)  # +max
            w1 = opool.tile([P, MT], f32)
            for _ in range(2):  # leaky_relu = max(z, 0.01 z)
                nc.vector.tensor_scalar_mul(w1[:], z[:], 0.01)
                nc.vector.tensor_tensor(z[:], z[:], w1[:], ALU.max)
            for _ in range(2):  # gelu(z) = 0.5 z (1 + erf(z/sqrt(2)))
                u = opool.tile([P, MT], f32, tag="u")
                nc.vector.tensor_scalar(
                    u[:], z[:], SQRT1_2, ERF_CLIP, ALU.mult, ALU.min
                )
                nc.vector.tensor_scalar_max(u[:], u[:], -ERF_CLIP)
                e = opool.tile([P, MT], f32, tag="e")
                nc.scalar.activation(e[:], u[:], AF.Erf)
                nc.vector.tensor_tensor(e[:], z[:], e[:], ALU.mult)
                nc.vector.tensor_tensor(z[:], z[:], e[:], ALU.add)
                nc.vector.tensor_scalar_mul(z[:], z[:], 0.5)
            z16 = opool.tile([P, MT], f16)
            nc.vector.tensor_copy(z16[:], z[:])

            # PE-transpose [128, MT] -> [MT, 128] (PE is idle by now) so the
            # final store writes 256B-contiguous DRAM runs per partition.
            ident = opool.tile([P, P], f16, name="ident")
            make_identity(nc, ident[:])
            # reuse a "ps" slot (same 2KB/partition footprint; all matmul use
            # of the tag is over by now)
            psT = pspool.tile([MT, 2 * FREE], f16, tag="ps", name="pst")
            nc.tensor.transpose(psT[:, :P], z16[:], ident[:])
            outT = opool.tile([MT, P], f16, name="outT")
            nc.vector.tensor_copy(outT[:], psT[:, :P])
            nc.sync.dma_start(out.rearrange("(t p) o -> t (p o)", p=P), outT[:])

    nc.compile()
    return nc


_prog_cache = {}
LAST_RESULTS = None


def kernel(x, W, bias):
    global LAST_RESULTS
    x = np.ascontiguousarray(x)
    W = np.ascontiguousarray(W)
    bias = np.ascontiguousarray(bias)
    assert x.shape == (M, K) and W.shape == (K, N) and bias.shape == (N,)

    key = (M_SHARD, N_CORES)
    if key not in _prog_cache:
        _prog_cache[key] = build_program(*key)
    nc = _prog_cache[key]

    shards = np.split(x, N_CORES, axis=0)
    in_maps = [{"x": s, "W": W, "bias": bias} for s in shards]
    res = run_bass_kernel_spmd(nc, in_maps, list(range(N_CORES)))
    LAST_RESULTS = res
    return np.concatenate([res.results[i]["out"] for i in range(N_CORES)], axis=0)


# revision 17
# speedup vs baseline: 1.1031x; 1.0101x over previous
"""Fused GEMM + bias + logsumexp + 2x leaky_relu + 2x exact-gelu kernel for TRN2.

Problem: x:(32768,2048)f16, W:(2048,2048)f16, bias:(2048,)f16
  y = x @ W + bias            (M, N)
  z = logsumexp(y, axis=1)    (M, 1)
  z = leaky_relu(leaky_relu(z, 0.01), 0.01)
  z = gelu(gelu(z, exact))    -> (M, 1) f16

Sharding: data-parallel over M across 8 cores (4096 rows each); W and bias
replicated. No cross-core communication; logsumexp reduces over N locally.

Per-core structure:
- W lives whole in SBUF (16 per-k chunk DMAs on the scalar HWDGE ring so the
  first matmuls only wait for chunk 0; the x DMA-transposes ride the sync ring
  in parallel — transposes are issued for super-block 0 *before* the W copies
  because Tile serializes transpose-after-copy, not copy-after-transpose).
- x arrives via DMA-transpose in 512-row super-blocks as 16 per-k xT tiles
  [128k x 512m], double-buffered.
- Per 128-row m-tile: 64 matmuls ([128,128]x[128,512] fp16, 16 k-steps x 4
  psum banks), then 4 fused DVE tensor_tensor_reduce ops that compute
  yneg = -(psum+bias) (f16) while min-accumulating negmax across banks, then
  one ACT Exp pass (scale=-1, bias=negmax) with accumulated row-sum.
- negmax/sumexp land in per-m-tile columns of [128, 32] stats tiles; the
  whole logsumexp tail (ln, +max, lrelu^2, erf-gelu^2) runs once, batched,
  at the end (keeps the ACT table on Exp for the entire main loop).
- The [128,32] result is block-transposed on DVE to [32,128] so the final
  store writes 256B-contiguous DRAM runs instead of 4096 scattered elements.
"""

import numpy as np

import concourse.bass as bass
import concourse.tile as tile
from concourse import bacc, mybir
from concourse.bass_utils import run_bass_kernel_spmd
from concourse.masks import make_identity

M, K, N = 32768, 2048, 2048
N_CORES = 8
M_SHARD = M // N_CORES  # 4096
P = 128
FREE = 512              # matmul moving free dim = one PSUM bank of f32
KT = K // P             # 16 k-subtiles
NB = N // FREE          # 4 psum banks per m-tile

f16 = mybir.dt.float16
f32 = mybir.dt.float32
AF = mybir.ActivationFunctionType
ALU = mybir.AluOpType

SQRT1_2 = 0.7071067811865476
ERF_CLIP = 5.9  # erf(5.9) == 1.0 to fp32 precision; clamp keeps ACT table in range


def build_program(m_shard=M_SHARD, num_devices=N_CORES, transposes_first=True):
    nc = bacc.Bacc(
        "TRN2",
        target_bir_lowering=False,
        debug=False,
        enable_asserts=False,
        num_devices=num_devices,
    )
    x = nc.dram_tensor("x", [m_shard, K], f16, kind="ExternalInput").ap()
    W = nc.dram_tensor("W", [K, N], f16, kind="ExternalInput").ap()
    bias = nc.dram_tensor("bias", [N], f16, kind="ExternalInput").ap()
    out = nc.dram_tensor("out", [m_shard, 1], f16, kind="ExternalOutput").ap()

    SBL = 512 if m_shard % 512 == 0 else P  # super-block rows per xT load
    MI = SBL // P                           # m-tiles per super-block
    NSB = m_shard // SBL                    # super-blocks
    MT = m_shard // P                       # total m-tiles

    with tile.TileContext(nc) as tc:
        with (
            tc.tile_pool(name="wpool", bufs=1) as wpool,
            tc.tile_pool(name="xpool", bufs=2) as xpool,
            tc.tile_pool(name="epool", bufs=2) as epool,
            tc.tile_pool(name="spool", bufs=4) as spool,
            tc.tile_pool(name="opool", bufs=1) as opool,
            tc.tile_pool(name="pspool", bufs=8, space="PSUM") as pspool,
        ):
            def issue_transposes(sb):
                xts = []
                for k in range(KT):
                    xk = xpool.tile([P, SBL], f16, tag=f"xk{k}", name=f"xT{sb}_{k}")
                    nc.sync.dma_start_transpose(
                        xk[:], x[bass.ds(sb * SBL, SBL), bass.ts(k, P)]
                    )
                    xts.append(xk)
                return xts

            # super-block 0 transposes FIRST (Tile serializes the first
            # transpose after a plain copy, but not the reverse). W and bias
            # go on the SWDGE (gpsimd) ring: it has its own completion
            # semaphore lanes, so the big W load doesn't interleave into the
            # 8 HWDGE lanes the transposes round-robin on.
            def issue_weight_loads():
                # bias first: it's tiny and the first m-tile epilogue needs it
                # long before W finishes streaming
                bias_sb = wpool.tile([P, N], f16, name="bias_sb")
                nc.gpsimd.dma_start(bias_sb[:], bias[None, :].to_broadcast((P, N)))
                W_sb = wpool.tile([P, KT, N], f16, name="W_sb")
                nc.gpsimd.dma_start(W_sb[:], W.rearrange("(ko p) n -> p ko n", p=P))
                return bias_sb, W_sb

            if transposes_first:
                xts = issue_transposes(0)
                bias_sb, W_sb = issue_weight_loads()
            else:
                bias_sb, W_sb = issue_weight_loads()
                xts = issue_transposes(0)

            nm_all = opool.tile([P, MT], f32)  # -rowmax per m-tile column
            se_all = opool.tile([P, MT], f32)  # sum(exp(y-max)) per m-tile column

            # identity for the final PE output-transpose; built early while
            # GPSIMD is otherwise idle
            ident = opool.tile([P, P], f16, name="ident")
            make_identity(nc, ident[:])

            for sb in range(NSB):
                if sb > 0:
                    xts = issue_transposes(sb)
                for mi in range(MI):
                    t = sb * MI + mi
                    pss = [
                        pspool.tile([P, FREE], f32, tag="ps", name=f"ps{t}_{nb}")
                        for nb in range(NB)
                    ]
                    for k in range(KT):
                        lhsT = xts[k][:, bass.ts(mi, P)]
                        for nb in range(NB):
                            nc.tensor.matmul(
                                pss[nb][:],
                                lhsT,
                                W_sb[:, k, bass.ts(nb, FREE)],
                                start=(k == 0),
                                stop=(k == KT - 1),
                            )
                    # y = psum + bias in f16 (matches the reference's fp16 GEMM
                    # output); negmax via negated row-max reduce.
                    # (tensor_tensor_reduce would fuse these but faults on HW.)
                    y = epool.tile([P, N], f16, tag="yneg", name=f"y{t}")
                    for nb in range(NB):
                        nc.vector.tensor_tensor(
                            y[:, bass.ts(nb, FREE)],
                            pss[nb][:],
                            bias_sb[:, bass.ts(nb, FREE)],
                            ALU.add,
                        )
                    nc.vector.reduce_max(
                        nm_all[:, t : t + 1],
                        y[:, :],
                        axis=mybir.AxisListType.X,
                        negate=True,
                    )
                    # exp(y - max); row-sum via the ACT accumulator
                    ejunk = epool.tile([P, N], f16, tag="ejunk", name=f"ejunk{t}")
                    nc.scalar.activation(
                        ejunk[:],
                        y[:],
                        AF.Exp,
                        bias=nm_all[:, t : t + 1],
                        accum_out=se_all[:, t : t + 1],
                    )

            # ---- batched tail over all MT m-tiles: [128, MT] ----
            z = opool.tile([P, MT], f32)
            nc.scalar.activation(z[:], se_all[:], AF.Ln)
            nc.vector.tensor_tensor(z[:], z[:], nm_all[:], ALU.subtract)  # +max
            w1 = opool.tile([P, MT], f32)
            for _ in range(2):  # leaky_relu = max(z, 0.01 z)
                nc.vector.tensor_scalar_mul(w1[:], z[:], 0.01)
                nc.vector.tensor_tensor(z[:], z[:], w1[:], ALU.max)
            for _ in range(2):  # gelu(z) = 0.5 z (1 + erf(z/sqrt(2)))
                u = opool.tile([P, MT], f32, tag="u")
                nc.vector.tensor_scalar(
                    u[:], z[:], SQRT1_2, ERF_CLIP, ALU.mult, ALU.min
                )
                nc.vector.tensor_scalar_max(u[:], u[:], -ERF_CLIP)
                e = opool.tile([P, MT], f32, tag="e")
                nc.scalar.activation(e[:], u[:], AF.Erf)
                nc.vector.tensor_tensor(e[:], z[:], e[:], ALU.mult)
                nc.vector.tensor_tensor(z[:], z[:], e[:], ALU.add)
                nc.vector.tensor_scalar_mul(z[:], z[:], 0.5)
            z16 = opool.tile([P, MT], f16)
            nc.vector.tensor_copy(z16[:], z[:])

            # PE-transpose [128, MT] -> [MT, 128] (PE is idle by now) so the
            # final store writes 256B-contiguous DRAM runs per partition.
            # reuse a "ps" slot (same 2KB/partition footprint; all matmul use
            # of the tag is over by now)
            psT = pspool.tile([MT, 2 * FREE], f16, tag="ps", name="pst")
            nc.tensor.transpose(psT[:, :P], z16[:], ident[:])
            outT = opool.tile([MT, P], f16, name="outT")
            nc.vector.tensor_copy(outT[:], psT[:, :P])
            nc.sync.dma_start(out.rearrange("(t p) o -> t (p o)", p=P), outT[:])

    nc.compile()
    return nc


_prog_cache = {}
LAST_RESULTS = None


def kernel(x, W, bias):
    global LAST_RESULTS
    x = np.ascontiguousarray(x)
    W = np.ascontiguousarray(W)
    bias = np.ascontiguousarray(bias)
    assert x.shape == (M, K) and W.shape == (K, N) and bias.shape == (N,)

    key = (M_SHARD, N_CORES)
    if key not in _prog_cache:
        _prog_cache[key] = build_program(*key)
    nc = _prog_cache[key]

    shards = np.split(x, N_CORES, axis=0)
    in_maps = [{"x": s, "W": W, "bias": bias} for s in shards]
    res = run_bass_kernel_spmd(nc, in_maps, list(range(N_CORES)))
    LAST_RESULTS = res
    return np.concatenate([res.results[i]["out"] for i in range(N_CORES)], axis=0)


# revision 19
# speedup vs baseline: 1.1406x; 1.0340x over previous
"""Fused GEMM + bias + logsumexp + 2x leaky_relu + 2x exact-gelu kernel for TRN2.

Problem: x:(32768,2048)f16, W:(2048,2048)f16, bias:(2048,)f16
  y = x @ W + bias            (M, N)
  z = logsumexp(y, axis=1)    (M, 1)
  z = leaky_relu(leaky_relu(z, 0.01), 0.01)
  z = gelu(gelu(z, exact))    -> (M, 1) f16

Sharding: data-parallel over M across 8 cores (4096 rows each); W and bias
replicated. No cross-core communication; logsumexp reduces over N locally.

Per-core structure:
- W lives whole in SBUF (16 per-k chunk DMAs on the scalar HWDGE ring so the
  first matmuls only wait for chunk 0; the x DMA-transposes ride the sync ring
  in parallel — transposes are issued for super-block 0 *before* the W copies
  because Tile serializes transpose-after-copy, not copy-after-transpose).
- x arrives via DMA-transpose in 512-row super-blocks as 16 per-k xT tiles
  [128k x 512m], double-buffered.
- Per 128-row m-tile: 64 matmuls ([128,128]x[128,512] fp16, 16 k-steps x 4
  psum banks), then 4 fused DVE tensor_tensor_reduce ops that compute
  yneg = -(psum+bias) (f16) while min-accumulating negmax across banks, then
  one ACT Exp pass (scale=-1, bias=negmax) with accumulated row-sum.
- negmax/sumexp land in per-m-tile columns of [128, 32] stats tiles; the
  whole logsumexp tail (ln, +max, lrelu^2, erf-gelu^2) runs once, batched,
  at the end (keeps the ACT table on Exp for the entire main loop).
- The [128,32] result is block-transposed on DVE to [32,128] so the final
  store writes 256B-contiguous DRAM runs instead of 4096 scattered elements.
"""

import numpy as np

import concourse.bass as bass
import concourse.tile as tile
from concourse import bacc, mybir
from concourse.bass_utils import run_bass_kernel_spmd
from concourse.masks import make_identity

M, K, N = 32768, 2048, 2048
N_CORES = 8
M_SHARD = M // N_CORES  # 4096
P = 128
FREE = 512              # matmul moving free dim = one PSUM bank of f32
KT = K // P             # 16 k-subtiles
NB = N // FREE          # 4 psum banks per m-tile

f16 = mybir.dt.float16
f32 = mybir.dt.float32
AF = mybir.ActivationFunctionType
ALU = mybir.AluOpType

SQRT1_2 = 0.7071067811865476
ERF_CLIP = 5.9  # erf(5.9) == 1.0 to fp32 precision; clamp keeps ACT table in range


def build_program(m_shard=M_SHARD, num_devices=N_CORES, transposes_first=True):
    nc = bacc.Bacc(
        "TRN2",
        target_bir_lowering=False,
        debug=False,
        enable_asserts=False,
        num_devices=num_devices,
    )
    x = nc.dram_tensor("x", [m_shard, K], f16, kind="ExternalInput").ap()
    W = nc.dram_tensor("W", [K, N], f16, kind="ExternalInput").ap()
    bias = nc.dram_tensor("bias", [N], f16, kind="ExternalInput").ap()
    out = nc.dram_tensor("out", [m_shard, 1], f16, kind="ExternalOutput").ap()

    SBL = 512 if m_shard % 512 == 0 else P  # super-block rows per xT load
    MI = SBL // P                           # m-tiles per super-block
    NSB = m_shard // SBL                    # super-blocks
    MT = m_shard // P                       # total m-tiles

    with tile.TileContext(nc) as tc:
        with (
            tc.tile_pool(name="wpool", bufs=1) as wpool,
            tc.tile_pool(name="xpool", bufs=2) as xpool,
            tc.tile_pool(name="epool", bufs=2) as epool,
            tc.tile_pool(name="spool", bufs=4) as spool,
            tc.tile_pool(name="opool", bufs=1) as opool,
            tc.tile_pool(name="pspool", bufs=8, space="PSUM") as pspool,
        ):
            def issue_transposes(sb):
                xts = []
                for k in range(KT):
                    xk = xpool.tile([P, SBL], f16, tag=f"xk{k}", name=f"xT{sb}_{k}")
                    nc.sync.dma_start_transpose(
                        xk[:], x[bass.ds(sb * SBL, SBL), bass.ts(k, P)]
                    )
                    xts.append(xk)
                return xts

            # Head: Tile serializes every copy<->transpose transition in the
            # scheduled DMA order, so W and DMA-transposes can never overlap.
            # Instead, super-block 0's x arrives as plain row-slab loads and
            # is transposed ON THE PE (which would otherwise idle while W
            # streams in); DMA-transposes only start at super-block 1, after
            # all head copies are done.
            xn = []
            for mi in range(MI):
                xnm = xpool.tile([P, K], f16, tag=f"xn{mi}", name=f"xn{mi}")
                nc.sync.dma_start(xnm[:], x[bass.ds(mi * P, P), :])
                xn.append(xnm)

            # bias first: tiny, and the first m-tile epilogue needs it long
            # before W finishes streaming. W in 4 k-chunks so early matmuls
            # only wait for their chunk. All on the SWDGE (gpsimd) ring: its
            # own completion-semaphore lanes, separate from HWDGE's 8.
            bias_sb = wpool.tile([P, N], f16, name="bias_sb")
            nc.gpsimd.dma_start(bias_sb[:], bias[None, :].to_broadcast((P, N)))
            W_view = W.rearrange("(ko p) n -> p ko n", p=P)
            KC = 4  # k-slices per W chunk
            Wcs = []
            for c in range(KT // KC):
                wc = wpool.tile([P, KC, N], f16, tag=f"Wc{c}", name=f"Wc{c}")
                nc.gpsimd.dma_start(wc[:], W_view[:, c * KC : (c + 1) * KC, :])
                Wcs.append(wc)

            nm_all = opool.tile([P, MT], f32)  # -rowmax per m-tile column
            se_all = opool.tile([P, MT], f32)  # sum(exp(y-max)) per m-tile column

            # identity for PE transposes; built early while GPSIMD is free
            ident = opool.tile([P, P], f16, name="ident")
            make_identity(nc, ident[:])

            # PE-transpose super-block 0: 64 [128,128] blocks through PSUM
            xts = []
            for k in range(KT):
                xk = xpool.tile([P, SBL], f16, tag=f"xk{k}", name=f"xT0_{k}")
                xts.append(xk)
            for mi in range(MI):
                for k in range(KT):
                    pt = pspool.tile(
                        [P, 2 * FREE], f16, tag="ps", name=f"pt{mi}_{k}"
                    )
                    nc.tensor.transpose(
                        pt[:, :P], xn[mi][:, bass.ts(k, P)], ident[:]
                    )
                    nc.vector.tensor_copy(xts[k][:, bass.ts(mi, P)], pt[:, :P])

            for sb in range(NSB):
                if sb > 0:
                    xts = issue_transposes(sb)
                for mi in range(MI):
                    t = sb * MI + mi
                    pss = [
                        pspool.tile([P, FREE], f32, tag="ps", name=f"ps{t}_{nb}")
                        for nb in range(NB)
                    ]
                    for k in range(KT):
                        lhsT = xts[k][:, bass.ts(mi, P)]
                        for nb in range(NB):
                            nc.tensor.matmul(
                                pss[nb][:],
                                lhsT,
                                Wcs[k // KC][:, k % KC, bass.ts(nb, FREE)],
                                start=(k == 0),
                                stop=(k == KT - 1),
                            )
                    # y = psum + bias in f16 (matches the reference's fp16 GEMM
                    # output); negmax via negated row-max reduce.
                    # (tensor_tensor_reduce would fuse these but faults on HW.)
                    y = epool.tile([P, N], f16, tag="yneg", name=f"y{t}")
                    for nb in range(NB):
                        nc.vector.tensor_tensor(
                            y[:, bass.ts(nb, FREE)],
                            pss[nb][:],
                            bias_sb[:, bass.ts(nb, FREE)],
                            ALU.add,
                        )
                    nc.vector.reduce_max(
                        nm_all[:, t : t + 1],
                        y[:, :],
                        axis=mybir.AxisListType.X,
                        negate=True,
                    )
                    # exp(y - max); row-sum via the ACT accumulator
                    ejunk = epool.tile([P, N], f16, tag="ejunk", name=f"ejunk{t}")
                    nc.scalar.activation(
                        ejunk[:],
                        y[:],
                        AF.Exp,
                        bias=nm_all[:, t : t + 1],
                        accum_out=se_all[:, t : t + 1],
                    )

            # ---- batched tail over all MT m-tiles: [128, MT] ----
            z = opool.tile([P, MT], f32)
            nc.scalar.activation(z[:], se_all[:], AF.Ln)
            nc.vector.tensor_tensor(z[:], z[:], nm_all[:], ALU.subtract)  # +max
            w1 = opool.tile([P, MT], f32)
            for _ in range(2):  # leaky_relu = max(z, 0.01 z)
                nc.vector.tensor_scalar_mul(w1[:], z[:], 0.01)
                nc.vector.tensor_tensor(z[:], z[:], w1[:], ALU.max)
            for _ in range(2):  # gelu(z) = 0.5 z (1 + erf(z/sqrt(2)))
                u = opool.tile([P, MT], f32, tag="u")
                nc.vector.tensor_scalar(
                    u[:], z[:], SQRT1_2, ERF_CLIP, ALU.mult, ALU.min
                )
                nc.vector.tensor_scalar_max(u[:], u[:], -ERF_CLIP)
                e = opool.tile([P, MT], f32, tag="e")
                nc.scalar.activation(e[:], u[:], AF.Erf)
                nc.vector.tensor_tensor(e[:], z[:], e[:], ALU.mult)
                nc.vector.tensor_tensor(z[:], z[:], e[:], ALU.add)
                nc.vector.tensor_scalar_mul(z[:], z[:], 0.5)
            z16 = opool.tile([P, MT], f16)
            nc.vector.tensor_copy(z16[:], z[:])

            # PE-transpose [128, MT] -> [MT, 128] (PE is idle by now) so the
            # final store writes 256B-contiguous DRAM runs per partition.
            # reuse a "ps" slot (same 2KB/partition footprint; all matmul use
            # of the tag is over by now)
            psT = pspool.tile([MT, 2 * FREE], f16, tag="ps", name="pst")
            nc.tensor.transpose(psT[:, :P], z16[:], ident[:])
            outT = opool.tile([MT, P], f16, name="outT")
            nc.vector.tensor_copy(outT[:], psT[:, :P])
            nc.sync.dma_start(out.rearrange("(t p) o -> t (p o)", p=P), outT[:])

    nc.compile()
    return nc


_prog_cache = {}
LAST_RESULTS = None


def kernel(x, W, bias):
    global LAST_RESULTS
    x = np.ascontiguousarray(x)
    W = np.ascontiguousarray(W)
    bias = np.ascontiguousarray(bias)
    assert x.shape == (M, K) and W.shape == (K, N) and bias.shape == (N,)

    key = (M_SHARD, N_CORES)
    if key not in _prog_cache:
        _prog_cache[key] = build_program(*key)
    nc = _prog_cache[key]

    shards = np.split(x, N_CORES, axis=0)
    in_maps = [{"x": s, "W": W, "bias": bias} for s in shards]
    res = run_bass_kernel_spmd(nc, in_maps, list(range(N_CORES)))
    LAST_RESULTS = res
    return np.concatenate([res.results[i]["out"] for i in range(N_CORES)], axis=0)
